# revision 29
# baseline (speedup 1.0000x reference)
"""Row-scale kernel: C = diag(A) @ B  (scale row i of B by A[i]).

Full shapes: A [16384] f32, B [16384, 4096] f32 -> C [16384, 4096] f32.
Sharding: pure data parallel over rows, 2048 rows per core on 8 cores.

Per-core layout: rows are interleaved over partitions, row r = p*T + t
(p = partition 0..127, t = tile 0..15).  That makes the per-tile scale
vector a_sb[:, t] a plain column of an A tile loaded with ONE contiguous
8 KiB DMA, and each B tile a clean 2D pattern (16 KiB contiguous per
partition, 256 KiB partition stride).

Raw Bass (no Tile framework) with an explicit software pipeline:
  SP sequencer   : B-tile loads  (HWDGE qSP ring)
  DVE            : per-partition scale multiply (in place)
  ACT sequencer  : C-tile stores (HWDGE qAct ring)
Per-buffer-slot semaphores; every instruction carries at most one
embedded wait (standalone sequencer waits otherwise) — the walrus
codegen rejects multi-wait TensorScalar instructions.
"""

import os

import numpy as np

import concourse.bass as bass
import concourse.mybir as mybir
from concourse.bass_utils import run_bass_kernel_spmd

N = 16384
M = 4096
N_CORES = 8
ROWS = N // N_CORES  # 2048 rows per core
P = 128              # SBUF partitions
T = ROWS // P        # 16 row-tiles per core
K = 8                # pipeline buffer slots (K * 16KiB = 128KiB per partition)

_nc_cache = {}
last_exec_time_ns = None


def _build_nc(reps=1, variant=0, ch=2, serialize=False, lead2=None, scratch=16384):
    """reps>1 repeats the whole kernel body back-to-back inside one NEFF
    (bench-only: isolates steady-state per-rep time from launch overhead);
    reps=0 builds an empty kernel (fixed-overhead measurement).
    Semaphore thresholds are cumulative over the global tile index g.

    variant 0: loads on SP ring, stores on ACT ring, 2 MiB tiles.
    variant 1: like 0 but paired tiles (4 MiB DMAs, two muls per slot).
    variant 2: loads split half/half across SP+ACT rings, stores on the
               gpsimd SWDGE queue.
    """
    nc = bass.Bass("TRN2", debug=False, dynamic_dma_scratch_size=scratch)
    A = nc.declare_dram_parameter("A", [ROWS], mybir.dt.float32, isOutput=False)
    B = nc.declare_dram_parameter("B", [ROWS, M], mybir.dt.float32, isOutput=False)
    C = nc.declare_dram_parameter("C", [ROWS, M], mybir.dt.float32, isOutput=True)

    if reps == 0:
        with nc.Block() as block:

            @block.sync
            def _(sync: bass.BassEngine):
                pass

        return nc

    # row r = p*T + t  (p outer, t inner) -> einops "(p t)"
    A2 = A.rearrange("(p t) -> p t", p=P)          # [128, 16]
    B3 = B.rearrange("(p t) m -> p t m", p=P)      # [128, 16, 4096]
    C3 = C.rearrange("(p t) m -> p t m", p=P)

    a_sb = nc.alloc_sbuf_tensor("a_sb", [P, T], mybir.dt.float32).ap()

    lda = nc.alloc_semaphore("lda")
    vs = nc.alloc_semaphore("vs")

    if variant in (0, 2, 4, 5, 6):
        work = nc.alloc_sbuf_tensor("work", [P, K * M], mybir.dt.float32).ap()

        def slot(k):
            return work[:, k * M : (k + 1) * M]

        ld = [nc.alloc_semaphore(f"ld{k}") for k in range(K)]
        st = [nc.alloc_semaphore(f"st{k}") for k in range(K)]
        G = reps * T  # total tile count across reps; data tile = g % T

    if variant == 0:
        with nc.Block() as block:

            @block.sync
            def _(sync: bass.BassEngine):
                sync.dma_start(out=a_sb, in_=A2).then_inc(lda, 16)
                for g in range(G):
                    t, k = g % T, g % K
                    if g >= K:
                        # slot free once store g-K fully landed
                        sync.wait_ge(st[k], 16 * (g // K))
                    sync.dma_start(out=slot(k), in_=B3[:, t, :]).then_inc(ld[k], 16)

            @block.vector
            def _(vector: bass.BassEngine):
                vector.wait_ge(lda, 16)
                for g in range(G):
                    t, k = g % T, g % K
                    vector.wait_ge(ld[k], 16 * (g // K + 1))
                    vector.tensor_scalar_mul(
                        slot(k), slot(k), a_sb[:, t : t + 1]
                    ).then_inc(vs, 1)

            @block.scalar
            def _(scalar: bass.BassEngine):
                for g in range(G):
                    t, k = g % T, g % K
                    scalar.wait_ge(vs, g + 1)
                    scalar.dma_start(out=C3[:, t, :], in_=slot(k)).then_inc(st[k], 16)
                # drain: all C writes must land before the end-of-kernel
                # barrier, else the NEFF can "complete" with stores in flight
                for k in range(K):
                    scalar.wait_ge(st[k], 16 * ((G - 1 - k) // K + 1))

    elif variant == 1:
        # paired tiles: one DMA covers data tiles (2j, 2j+1) -> 4 MiB
        KP = K // 2  # slots of 2*M floats
        TP = T // 2  # 8 paired tiles per rep
        work = nc.alloc_sbuf_tensor("work", [P, KP * 2 * M], mybir.dt.float32).ap()

        def pslot(k):
            return work[:, k * 2 * M : (k + 1) * 2 * M]

        ld = [nc.alloc_semaphore(f"ld{k}") for k in range(KP)]
        st = [nc.alloc_semaphore(f"st{k}") for k in range(KP)]
        G = reps * TP
        B4 = B.rearrange("(p j u) m -> p j (u m)", p=P, u=2)  # [128, 8, 8192]
        C4 = C.rearrange("(p j u) m -> p j (u m)", p=P, u=2)

        with nc.Block() as block:

            @block.sync
            def _(sync: bass.BassEngine):
                sync.dma_start(out=a_sb, in_=A2).then_inc(lda, 16)
                for g in range(G):
                    j, k = g % TP, g % KP
                    if g >= KP:
                        sync.wait_ge(st[k], 16 * (g // KP))
                    sync.dma_start(out=pslot(k), in_=B4[:, j, :]).then_inc(ld[k], 16)

            @block.vector
            def _(vector: bass.BassEngine):
                vector.wait_ge(lda, 16)
                for g in range(G):
                    j, k = g % TP, g % KP
                    vector.wait_ge(ld[k], 16 * (g // KP + 1))
                    s = pslot(k)
                    vector.tensor_scalar_mul(
                        s[:, :M], s[:, :M], a_sb[:, 2 * j : 2 * j + 1]
                    )
                    vector.tensor_scalar_mul(
                        s[:, M:], s[:, M:], a_sb[:, 2 * j + 1 : 2 * j + 2]
                    ).then_inc(vs, 1)

            @block.scalar
            def _(scalar: bass.BassEngine):
                for g in range(G):
                    j, k = g % TP, g % KP
                    scalar.wait_ge(vs, g + 1)
                    scalar.dma_start(out=C4[:, j, :], in_=pslot(k)).then_inc(st[k], 16)
                for k in range(KP):
                    scalar.wait_ge(st[k], 16 * ((G - 1 - k) // KP + 1))

    elif variant == 2:
        # loads: left half on SP ring, right half on ACT ring; stores SWDGE
        H = M // 2
        ldr = [nc.alloc_semaphore(f"ldr{k}") for k in range(K)]

        with nc.Block() as block:

            @block.sync
            def _(sync: bass.BassEngine):
                sync.dma_start(out=a_sb, in_=A2).then_inc(lda, 16)
                for g in range(G):
                    t, k = g % T, g % K
                    if g >= K:
                        sync.wait_ge(st[k], 16 * (g // K))
                    sync.dma_start(
                        out=slot(k)[:, :H], in_=B3[:, t, :H]
                    ).then_inc(ld[k], 16)

            @block.scalar
            def _(scalar: bass.BassEngine):
                for g in range(G):
                    t, k = g % T, g % K
                    if g >= K:
                        scalar.wait_ge(st[k], 16 * (g // K))
                    scalar.dma_start(
                        out=slot(k)[:, H:], in_=B3[:, t, H:]
                    ).then_inc(ldr[k], 16)

            @block.vector
            def _(vector: bass.BassEngine):
                vector.wait_ge(lda, 16)
                for g in range(G):
                    t, k = g % T, g % K
                    vector.wait_ge(ld[k], 16 * (g // K + 1))
                    vector.wait_ge(ldr[k], 16 * (g // K + 1))
                    vector.tensor_scalar_mul(
                        slot(k), slot(k), a_sb[:, t : t + 1]
                    ).then_inc(vs, 1)

            @block.gpsimd
            def _(gpsimd: bass.BassEngine):
                for g in range(G):
                    t, k = g % T, g % K
                    gpsimd.wait_ge(vs, g + 1)
                    gpsimd.dma_start(out=C3[:, t, :], in_=slot(k)).then_inc(st[k], 16)
                for k in range(K):
                    gpsimd.wait_ge(st[k], 16 * ((G - 1 - k) // K + 1))

    elif variant == 3:
        # v0 with finer tiles: each [128, M] row-tile split into `ch` column
        # chunks, each chunk its own pipeline slot (one outstanding DMA per
        # slot semaphore — required for cumulative thresholds to be safe).
        # `serialize` drains the pipe between reps (bench: isolates true
        # single-exec body time including ramp head/tail).
        CH = ch
        W = M // CH
        K3 = min(2 * K * CH, 160 * 1024 // (W * 4), 24)  # slots of width W
        work3 = nc.alloc_sbuf_tensor("work3", [P, K3 * W], mybir.dt.float32).ap()

        def slot3(k):
            return work3[:, k * W : (k + 1) * W]

        ld3 = [nc.alloc_semaphore(f"l3_{k}") for k in range(K3)]
        st3 = [nc.alloc_semaphore(f"s3_{k}") for k in range(K3)]

        ld_cnt = [0] * K3
        st_cnt = [0] * K3
        vs_cnt = 0
        load_plan, mul_plan, store_plan = [], [], []
        g = 0
        for rep in range(reps):
            if serialize and rep > 0:
                load_plan.append(("drain", list(st_cnt)))
            for tt in range(T):
                t = tt
                for c in range(CH):
                    k = g % K3
                    wait_st = None
                    if g >= K3:
                        wait_st = (k, st_cnt[k])
                    ld_cnt[k] += 16
                    load_plan.append(("load", t, c, k, wait_st))
                    mul_plan.append((t, c, k, ld_cnt[k]))
                    vs_cnt += 1
                    st_cnt[k] += 16
                    store_plan.append((t, c, k, vs_cnt))
                    g += 1
        final_st = list(st_cnt)

        with nc.Block() as block:

            @block.sync
            def _(sync: bass.BassEngine):
                sync.dma_start(out=a_sb, in_=A2).then_inc(lda, 16)
                for item in load_plan:
                    if item[0] == "drain":
                        for k, v in enumerate(item[1]):
                            if v:
                                sync.wait_ge(st3[k], v)
                        continue
                    _, t, c, k, wait_st = item
                    if wait_st is not None:
                        sync.wait_ge(st3[wait_st[0]], wait_st[1])
                    cols = slice(c * W, (c + 1) * W)
                    sync.dma_start(out=slot3(k), in_=B3[:, t, cols]).then_inc(
                        ld3[k], 16
                    )

            @block.vector
            def _(vector: bass.BassEngine):
                vector.wait_ge(lda, 16)
                for t, c, k, ld_thresh in mul_plan:
                    vector.wait_ge(ld3[k], ld_thresh)
                    vector.tensor_scalar_mul(
                        slot3(k), slot3(k), a_sb[:, t : t + 1]
                    ).then_inc(vs, 1)

            @block.scalar
            def _(scalar: bass.BassEngine):
                for t, c, k, vs_thresh in store_plan:
                    cols = slice(c * W, (c + 1) * W)
                    scalar.wait_ge(vs, vs_thresh)
                    scalar.dma_start(out=C3[:, t, cols], in_=slot3(k)).then_inc(
                        st3[k], 16
                    )
                for k in range(K3):
                    if final_st[k]:
                        scalar.wait_ge(st3[k], final_st[k])

    elif variant == 4:
        # Phase-alternated bursts: HBM does either reads or writes, never
        # mixed (solo read 352 GB/s + solo write 380 GB/s beats mixed 325).
        # Burst = S consecutive tiles.  Stores of burst i gate on ALL loads
        # of burst i done; loads of burst i+1 gate on ALL stores of burst i.
        S = K  # burst size = slot count (each burst fills all slots)
        G = reps * T
        assert G % S == 0
        ld_cnt = [0] * K
        st_cnt = [0] * K
        with nc.Block() as block:

            @block.sync
            def _(sync: bass.BassEngine):
                cnt = [0] * K
                sync.dma_start(out=a_sb, in_=A2).then_inc(lda, 16)
                for g in range(G):
                    t, k = g % T, g % K
                    if g >= S and g % S == 0:
                        # R-burst starts only after previous W-burst drained
                        for kk in range(K):
                            sync.wait_ge(st[kk], 16 * (g // K))
                    cnt[k] += 16
                    sync.dma_start(out=slot(k), in_=B3[:, t, :]).then_inc(ld[k], 16)

            @block.vector
            def _(vector: bass.BassEngine):
                cnt = [0] * K
                vector.wait_ge(lda, 16)
                for g in range(G):
                    t, k = g % T, g % K
                    cnt[k] += 16
                    vector.wait_ge(ld[k], cnt[k])
                    vector.tensor_scalar_mul(
                        slot(k), slot(k), a_sb[:, t : t + 1]
                    ).then_inc(vs, 1)

            @block.scalar
            def _(scalar: bass.BassEngine):
                cnt = [0] * K
                for g in range(G):
                    t, k = g % T, g % K
                    if g % S == 0:
                        # W-burst starts only after this R-burst fully landed
                        for kk in range(K):
                            scalar.wait_ge(ld[kk], 16 * (g // K + 1))
                    cnt[k] += 16
                    scalar.wait_ge(vs, g + 1)
                    scalar.dma_start(out=C3[:, t, :], in_=slot(k)).then_inc(st[k], 16)
                for k in range(K):
                    scalar.wait_ge(st[k], 16 * ((G - 1 - k) // K + 1))

    elif variant == 5:
        # Soft phase alternation: per-tile correctness waits as v0, plus
        # burst-shaping gates with LEAD tiles of overlap at each transition.
        # ch parameter is reused as LEAD (bench tokens like 5:2).
        LEAD = ch        # R->W overlap (store-burst gate)
        LEAD2 = lead2 if lead2 is not None else ch  # W->R overlap (load gate)
        S = K
        G = reps * T
        with nc.Block() as block:

            @block.sync
            def _(sync: bass.BassEngine):
                sync.dma_start(out=a_sb, in_=A2).then_inc(lda, 16)
                for g in range(G):
                    t, k = g % T, g % K
                    if g >= S and g % S == 0:
                        # shaping: most of previous W-burst done
                        kk = S - 1 - LEAD2
                        sync.wait_ge(st[kk], 16 * (g // K))
                    if g >= K:
                        sync.wait_ge(st[k], 16 * (g // K))  # slot free
                    sync.dma_start(out=slot(k), in_=B3[:, t, :]).then_inc(ld[k], 16)

            @block.vector
            def _(vector: bass.BassEngine):
                vector.wait_ge(lda, 16)
                for g in range(G):
                    t, k = g % T, g % K
                    vector.wait_ge(ld[k], 16 * (g // K + 1))
                    vector.tensor_scalar_mul(
                        slot(k), slot(k), a_sb[:, t : t + 1]
                    ).then_inc(vs, 1)

            @block.scalar
            def _(scalar: bass.BassEngine):
                for g in range(G):
                    t, k = g % T, g % K
                    if g % S == 0:
                        # shaping: most of this R-burst done (clamped for a
                        # partial final burst)
                        kk = S - 1 - LEAD
                        limit = min(g + S, G)
                        n_loads = ((limit - 1 - kk) // K + 1) if kk < limit else 0
                        if n_loads:
                            scalar.wait_ge(ld[kk], 16 * n_loads)
                    scalar.wait_ge(vs, g + 1)
                    scalar.dma_start(out=C3[:, t, :], in_=slot(k)).then_inc(st[k], 16)
                for k in range(K):
                    scalar.wait_ge(st[k], 16 * ((G - 1 - k) // K + 1))

    elif variant == 6:
        # v5 (soft phase alternation) + sequential row mapping: row r = t*P+p,
        # so B/C tiles are fully-contiguous 2 MiB regions (contiguous C writes
        # measured ~3.5% faster than the interleaved pattern).  A is loaded
        # once with a strided transpose AP on the ACT ring (overlaps R-phase).
        LEAD = ch
        LEAD2 = lead2 if lead2 is not None else ch
        S = K
        G = reps * T
        Bseq = B.rearrange("(t p) m -> t p m", p=P)   # [16, 128, 4096]
        Cseq = C.rearrange("(t p) m -> t p m", p=P)
        A2T = A.rearrange("(t p) -> p t", p=P)        # a_sb[p,t] = A[t*P+p]

        with nc.Block() as block:

            @block.sync
            def _(sync: bass.BassEngine):
                for g in range(G):
                    t, k = g % T, g % K
                    if g >= S and g % S == 0:
                        kk = S - 1 - LEAD2
                        sync.wait_ge(st[kk], 16 * (g // K))
                    if g >= K:
                        sync.wait_ge(st[k], 16 * (g // K))  # slot free
                    sync.dma_start(out=slot(k), in_=Bseq[t]).then_inc(ld[k], 16)

            @block.vector
            def _(vector: bass.BassEngine):
                vector.wait_ge(lda, 16)
                for g in range(G):
                    t, k = g % T, g % K
                    vector.wait_ge(ld[k], 16 * (g // K + 1))
                    vector.tensor_scalar_mul(
                        slot(k), slot(k), a_sb[:, t : t + 1]
                    ).then_inc(vs, 1)

            @block.scalar
            def _(scalar: bass.BassEngine):
                with nc.allow_non_contiguous_dma(
                    reason="one-time 8KB transposed A load"
                ):
                    scalar.dma_start(out=a_sb, in_=A2T).then_inc(lda, 16)
                for g in range(G):
                    t, k = g % T, g % K
                    if g % S == 0:
                        kk = S - 1 - LEAD
                        limit = min(g + S, G)
                        n_loads = ((limit - 1 - kk) // K + 1) if kk < limit else 0
                        if n_loads:
                            scalar.wait_ge(ld[kk], 16 * n_loads)
                    scalar.wait_ge(vs, g + 1)
                    scalar.dma_start(out=Cseq[t], in_=slot(k)).then_inc(st[k], 16)
                for k in range(K):
                    scalar.wait_ge(st[k], 16 * ((G - 1 - k) // K + 1))

    elif variant in (8, 9, 10, 11, 12):
        # BW microbenches (bench-only, output is garbage — no correctness):
        # 8 = read-only, 9 = write-only, 10 = uncoupled read+write
        # (interleaved row mapping: per-partition 16KB blocks strided 256KB);
        # 11/12 = read-only/write-only with fully-contiguous 2MiB tiles.
        NS = 8
        workm = nc.alloc_sbuf_tensor("workm", [P, K * M], mybir.dt.float32).ap()

        def mslot(k):
            return workm[:, k * M : (k + 1) * M]

        Bseq = B.rearrange("(t p) m -> t p m", p=P)  # [16, 128, 4096] contiguous
        Cseq = C.rearrange("(t p) m -> t p m", p=P)

        lds = [nc.alloc_semaphore(f"ml{i}") for i in range(NS)]
        sts = [nc.alloc_semaphore(f"ms{i}") for i in range(NS)]
        G = reps * T
        ld_tot = [0] * NS
        st_tot = [0] * NS
        for g in range(G):
            ld_tot[g % NS] += 16
            st_tot[g % NS] += 16

        with nc.Block() as block:
            if variant in (8, 10, 11):

                @block.sync
                def _(sync: bass.BassEngine):
                    for g in range(G):
                        t = g % T
                        src = B3[:, t, :] if variant != 11 else Bseq[t]
                        sync.dma_start(out=mslot(g % K), in_=src).then_inc(
                            lds[g % NS], 16
                        )
                    for i in range(NS):
                        if ld_tot[i]:
                            sync.wait_ge(lds[i], ld_tot[i])

            if variant in (9, 10, 12):

                @block.scalar
                def _(scalar: bass.BassEngine):
                    for g in range(G):
                        t = g % T
                        dst = C3[:, t, :] if variant != 12 else Cseq[t]
                        scalar.dma_start(out=dst, in_=mslot(g % K)).then_inc(
                            sts[g % NS], 16
                        )
                    for i in range(NS):
                        if st_tot[i]:
                            scalar.wait_ge(sts[i], st_tot[i])

    else:
        raise ValueError(variant)

    return nc


def kernel(A, B):
    global last_exec_time_ns
    A = np.ascontiguousarray(np.asarray(A), dtype=np.float32)
    B = np.ascontiguousarray(np.asarray(B), dtype=np.float32)
    assert A.shape == (N,) and B.shape == (N, M)

    if "nc" not in _nc_cache:
        _nc_cache["nc"] = _build_nc()
    nc = _nc_cache["nc"]

    in_maps = [
        {"A": A[c * ROWS : (c + 1) * ROWS], "B": B[c * ROWS : (c + 1) * ROWS]}
        for c in range(N_CORES)
    ]
    trace = bool(os.environ.get("BASS_KERNEL_TRACE"))
    res = run_bass_kernel_spmd(nc, in_maps, list(range(N_CORES)), trace=trace)
    last_exec_time_ns = res.exec_time_ns
    return np.concatenate([res.results[c]["C"] for c in range(N_CORES)], axis=0)
